# revision 1
# baseline (speedup 1.0000x reference)
"""MoE (top-2 of 8 experts, SwiGLU FFN) for 8 Trainium2 NeuronCores.

Strategy: expert parallelism. The router / RMSNorm / top-k dispatch are
O(T*D) host-side numpy; each NeuronCore runs the SwiGLU FFN of one expert
over the tokens routed to it (gathered + padded to a fixed capacity C).
All matmuls run in bf16 with fp32 PSUM accumulation, feature-major layout:

  core e computes   G.T = Wg.T @ Xg.T   [F, C]   (lhsT = Wg, natural layout)
                    U.T = Wu.T @ Xg.T   [F, C]
                    H.T = silu(G.T) * U.T
                    Y.T = Wd.T @ H.T    [D, C]   (lhsT = Wd, natural layout)

The host scales rows of Y by the renormalized top-2 softmax weight and
scatter-adds into the output.
"""

import numpy as np
import ml_dtypes

import concourse.bass as bass  # noqa: F401  (engine handles live on nc)
import concourse.mybir as mybir
import concourse.tile as tile
from concourse import bacc, bass_utils

EPS = 1e-6
TOP_K = 2
N_CORES = 8
P = 128

# Set by a test harness to capture profile info; default path is untouched.
TRACE = False
TRACE_KWARGS = {}
LAST_RESULTS = None

_PROG_CACHE = {}


def _build_program(C, D, F):
    """One-expert SwiGLU FFN over C tokens: yt[D,C] = ffn(xgt[D,C]) (transposed)."""
    assert C % 256 == 0
    NT = C // 2  # token chunk (2 chunks so H fits SBUF)
    ns_list = []
    o = 0
    while o < NT:
        nn = min(512, NT - o)
        ns_list.append((o, nn))
        o += nn
    KD = D // P   # contraction tiles over D (gate/up)
    KF = F // P   # contraction tiles over F (down)
    MF = F // P   # output F tiles (gate/up)
    MD = D // P   # output D tiles (down)
    bf = mybir.dt.bfloat16
    f32 = mybir.dt.float32
    AF = mybir.ActivationFunctionType

    nc = bacc.Bacc("TRN2", target_bir_lowering=False, debug=False)
    xgt = nc.dram_tensor("xgt", [D, C], bf, kind="ExternalInput").ap()
    wg = nc.dram_tensor("wg", [D, F], bf, kind="ExternalInput").ap()
    wu = nc.dram_tensor("wu", [D, F], bf, kind="ExternalInput").ap()
    wd = nc.dram_tensor("wd", [F, D], bf, kind="ExternalInput").ap()
    yt = nc.dram_tensor("yt", [D, C], f32, kind="ExternalOutput").ap()

    with tile.TileContext(nc) as tc:
        with (
            tc.tile_pool(name="xg", bufs=1) as xg_pool,
            tc.tile_pool(name="h", bufs=1) as h_pool,
            tc.tile_pool(name="wgu", bufs=4) as wgu_pool,
            tc.tile_pool(name="wdp", bufs=2) as wd_pool,
            tc.tile_pool(name="sg", bufs=4) as sg_pool,
            tc.tile_pool(name="ot", bufs=4) as o_pool,
            tc.tile_pool(name="ps", bufs=8, space="PSUM") as ps_pool,
        ):
            # whole gathered token block, K(D) on partitions: [128, KD, C]
            xg_t = xg_pool.tile([P, KD, C], bf)
            for k in range(KD):
                nc.sync.dma_start(xg_t[:, k, :], xgt[k * P:(k + 1) * P, :])

            for ci in range(2):
                c0 = ci * NT
                h_t = h_pool.tile([P, MF, NT], bf, tag="h")

                # ---- gate/up projections + silu*mul -> H.T chunk ----
                for mg in range(MF // 2):
                    wg_t = wgu_pool.tile([P, KD, 256], bf, tag="wgu")
                    wu_t = wgu_pool.tile([P, KD, 256], bf, tag="wgu")
                    for k in range(KD):
                        nc.sync.dma_start(
                            wg_t[:, k, :],
                            wg[k * P:(k + 1) * P, mg * 256:(mg + 1) * 256])
                        nc.sync.dma_start(
                            wu_t[:, k, :],
                            wu[k * P:(k + 1) * P, mg * 256:(mg + 1) * 256])
                    for mi in range(2):
                        m = mg * 2 + mi
                        for (n0, nn) in ns_list:
                            ps_g = ps_pool.tile([P, 512], f32, tag="ps")
                            ps_u = ps_pool.tile([P, 512], f32, tag="ps")
                            for k in range(KD):
                                nc.tensor.matmul(
                                    ps_g[:, :nn],
                                    wg_t[:, k, mi * P:(mi + 1) * P],
                                    xg_t[:, k, c0 + n0:c0 + n0 + nn],
                                    start=(k == 0), stop=(k == KD - 1))
                            for k in range(KD):
                                nc.tensor.matmul(
                                    ps_u[:, :nn],
                                    wu_t[:, k, mi * P:(mi + 1) * P],
                                    xg_t[:, k, c0 + n0:c0 + n0 + nn],
                                    start=(k == 0), stop=(k == KD - 1))
                            sg_t = sg_pool.tile([P, 512], f32, tag="sg")
                            nc.scalar.activation(sg_t[:, :nn], ps_g[:, :nn], AF.Silu)
                            nc.vector.tensor_mul(
                                h_t[:, m, n0:n0 + nn], sg_t[:, :nn], ps_u[:, :nn])

                # ---- down projection -> Y.T chunk ----
                for mgd in range(MD // 2):
                    wd_t = wd_pool.tile([P, KF, 256], bf, tag="wd")
                    for k in range(KF):
                        nc.sync.dma_start(
                            wd_t[:, k, :],
                            wd[k * P:(k + 1) * P, mgd * 256:(mgd + 1) * 256])
                    for mi in range(2):
                        m = mgd * 2 + mi
                        for (n0, nn) in ns_list:
                            ps_d = ps_pool.tile([P, 512], f32, tag="ps")
                            for k in range(KF):
                                nc.tensor.matmul(
                                    ps_d[:, :nn],
                                    wd_t[:, k, mi * P:(mi + 1) * P],
                                    h_t[:, k, n0:n0 + nn],
                                    start=(k == 0), stop=(k == KF - 1))
                            o_t = o_pool.tile([P, 512], f32, tag="ot")
                            nc.vector.tensor_copy(o_t[:, :nn], ps_d[:, :nn])
                            nc.sync.dma_start(
                                yt[m * P:(m + 1) * P, c0 + n0:c0 + n0 + nn],
                                o_t[:, :nn])

    nc.compile()
    return nc


def _get_program(C, D, F):
    key = (C, D, F)
    if key not in _PROG_CACHE:
        _PROG_CACHE[key] = _build_program(C, D, F)
    return _PROG_CACHE[key]


def kernel(hidden_states, ln_weight, w_router, w_gate, w_up, w_down):
    global LAST_RESULTS
    hs = np.asarray(hidden_states, dtype=np.float32)
    ln_w = np.asarray(ln_weight, dtype=np.float32)
    w_r = np.asarray(w_router, dtype=np.float32)
    w_gate = np.asarray(w_gate)
    w_up = np.asarray(w_up)
    w_down = np.asarray(w_down)

    B, S, D = hs.shape
    T = B * S
    E, _, F = w_gate.shape
    bf = ml_dtypes.bfloat16

    # ---- host: RMSNorm + router + top-2 dispatch (O(T*D), exact fp32) ----
    x = hs.reshape(T, D)
    var = np.mean(x * x, axis=-1, keepdims=True)
    xn = x * (1.0 / np.sqrt(var + EPS)) * ln_w
    router_logits = xn @ w_r                      # [T, E]
    lm = router_logits.max(-1, keepdims=True)
    probs = np.exp(router_logits - lm)
    probs /= probs.sum(-1, keepdims=True)
    top_idx = np.argpartition(-probs, TOP_K - 1, axis=-1)[:, :TOP_K]  # [T, k]
    top_vals = np.take_along_axis(probs, top_idx, axis=-1)
    top_vals = top_vals / top_vals.sum(-1, keepdims=True)

    flat_expert = top_idx.ravel()
    flat_token = np.repeat(np.arange(T, dtype=np.int64), TOP_K)
    flat_w = top_vals.ravel().astype(np.float32)
    counts = np.bincount(flat_expert, minlength=E)
    C = max(256, int(np.ceil(counts.max() / 256.0)) * 256)

    xnT_b = np.ascontiguousarray(xn.T).astype(bf)  # [D, T]

    rows_per_e = []
    w_per_e = []
    in_maps = []
    for e in range(E):
        sel = flat_expert == e
        rows = flat_token[sel]
        rows_per_e.append(rows)
        w_per_e.append(flat_w[sel])
        xgt = np.zeros((D, C), dtype=bf)
        xgt[:, :len(rows)] = xnT_b[:, rows]
        in_maps.append({
            "xgt": xgt,
            "wg": np.ascontiguousarray(w_gate[e]).astype(bf),
            "wu": np.ascontiguousarray(w_up[e]).astype(bf),
            "wd": np.ascontiguousarray(w_down[e]).astype(bf),
        })

    # ---- device: per-expert SwiGLU FFN on its own core ----
    nc = _get_program(C, D, F)
    res = bass_utils.run_bass_kernel_spmd(
        nc, in_maps, core_ids=list(range(N_CORES)),
        trace=TRACE, **TRACE_KWARGS)
    LAST_RESULTS = res

    # ---- host: combine-weight scale + scatter-add ----
    out = np.zeros((T, D), dtype=np.float32)
    for e in range(E):
        rows = rows_per_e[e]
        n_e = len(rows)
        if n_e == 0:
            continue
        y = res.results[e]["yt"][:, :n_e].T          # [n_e, D] fp32
        out[rows] += w_per_e[e][:, None] * y
    return out.reshape(B, S, D), router_logits
